# revision 9
# baseline (speedup 1.0000x reference)
"""Multi-head causal self-attention on 8 Trainium2 NeuronCores.

Problem: B=2, S=2048, D=1024, H=16 heads (dk=64), causal softmax attention,
fp32 in/out.  y = softmax(mask(Q K^T / sqrt(dk))) V  projected by Wo.

Sharding (no device-to-device communication needed):
  core c -> batch b = c // 4, head-group hg = c % 4 (4 heads = 256 dims each).
  Each core computes its 4 heads' attention output and a *partial* final
  projection (its 256 rows of the Wo contraction).  The host sums the 4
  partials per batch and stacks the 2 batches.

On-device layout strategy (per core):
  All matmul operands live in SBUF with the contraction dim on partitions.
  The host passes x^T and W^T slices so no on-device transposes are needed.
  Scores are computed transposed (S^T = K Q^T, keys on partitions) so that
  P^T = exp(S^T) is directly the stationary-side layout the PV matmul needs;
  softmax normalization moves to the *output* side (divide the 64-dim head
  output by the row sum), with the row sums produced for free by an extra
  ones-column appended to V.  All matmuls run as float32r (full PE rate);
  causal masking is tile-level: strictly-upper tiles are skipped entirely,
  diagonal tiles are zeroed post-exp with a gpsimd affine_select.  Heads are
  processed in pairs living on partitions 0-63 / 64-127 so the K=64 score
  matmuls pack into the PE array's row-tiling (concurrent execution).
"""

import sys

sys.path.insert(0, "/opt/trn_rl_repo")

import numpy as np

import concourse.bacc as bacc
import concourse.mybir as mybir
import concourse.tile as tile
from concourse.bass_utils import run_bass_kernel_spmd

F32 = mybir.dt.float32
F32R = mybir.dt.float32r
EXPF = mybir.ActivationFunctionType.Exp

B, S, D, H = 2, 2048, 1024, 16
DK = D // H          # 64
E = 256              # head dims per core (4 heads)
HL = 4               # local heads per core
QC = 512             # query chunk (free dim of S^T tiles)
NKT = S // 128       # 16 key tiles
NQC = S // QC        # 4
NDT = D // 128       # 8 contraction tiles for projections
SCALE = float(1.0 / np.sqrt(np.float32(DK)))


def _build_nc():
    nc = bacc.Bacc("TRN2", target_bir_lowering=False, debug=False)

    xT = nc.dram_tensor("xT", [D, S], F32R, kind="ExternalInput")
    wqT = nc.dram_tensor("wqT", [D, E], F32R, kind="ExternalInput")
    wkT = nc.dram_tensor("wkT", [D, E], F32R, kind="ExternalInput")
    wvT = nc.dram_tensor("wvT", [D, E], F32R, kind="ExternalInput")
    woT = nc.dram_tensor("woT", [E, D], F32R, kind="ExternalInput")
    out = nc.dram_tensor("out", [S, D], F32, kind="ExternalOutput")

    with tile.TileContext(nc) as tc:
        with (
            tc.tile_pool(name="const", bufs=1) as const,
            tc.tile_pool(name="work", bufs=6) as work,
            tc.tile_pool(name="outp", bufs=4) as outp,
            tc.tile_pool(name="norm", bufs=3) as norm,
            tc.tile_pool(name="psmm", bufs=5, space="PSUM") as psmm,
            tc.tile_pool(name="pspv", bufs=3, space="PSUM") as pspv,
        ):
            # ---- resident tensors -------------------------------------
            xT_sb = const.tile([128, NDT, S], F32R, tag="xT")
            wq_sb = const.tile([128, NDT, E], F32R, tag="wq")
            wk_sb = const.tile([128, NDT, E], F32R, tag="wk")
            wv_sb = const.tile([128, NDT, E], F32R, tag="wv")
            wo_sb = const.tile([128, 2, D], F32R, tag="wo")
            qT_sb = const.tile([128, 2, S], F32R, tag="qT")
            kT_sb = const.tile([128, 2, S], F32R, tag="kT")
            v_sb = const.tile([128, NKT, HL * (DK + 1)], F32R, tag="v")
            oT_sb = const.tile([128, 2, S], F32R, tag="oT")
            zeros = const.tile([128, HL], F32, tag="zeros")
            nc.vector.memset(zeros[:], 0.0)

            # ---- DMA in (per contraction tile so matmuls start early) --
            for w_sb, w_dr in ((wk_sb, wkT), (wq_sb, wqT), (wv_sb, wvT)):
                nc.sync.dma_start(w_sb[:], w_dr.rearrange("(k p) e -> p k e", p=128))
            xT_r = xT.rearrange("(k p) s -> p k s", p=128)
            for kt in range(NDT):
                nc.sync.dma_start(xT_sb[:, kt, :], xT_r[:, kt, :])
            nc.sync.dma_start(wo_sb[:], woT.rearrange("(g p) e -> p g e", p=128))

            # ---- K^T / Q^T projections --------------------------------
            # [e_local (2x128 part groups), s] = W^T.T @ x^T
            # Split each contraction chain in half (x^T tiles 0-3 / 4-7) so
            # the first half's matmuls run while the second half of x^T is
            # still arriving over DMA; halves are combined with an in-place
            # DVE add into the fp32r destination.  Only the chains attention
            # chunk 0 needs are emitted eagerly; the rest become deferred
            # PE filler drained inside the attention loop.
            def emit_proj(nm, w_sb, t_sb, g, sc, hf):
                k0, k1 = (0, NDT // 2) if hf == 0 else (NDT // 2, NDT)
                tgt = t_sb[:, g, sc * QC:(sc + 1) * QC]
                ps = psmm.tile([128, QC], F32, tag="mm",
                               name=f"pj{hf}_{nm}_{g}_{sc}")
                for kt in range(k0, k1):
                    nc.tensor.matmul(
                        ps[:],
                        w_sb[:, kt, g * 128:(g + 1) * 128],
                        xT_sb[:, kt, sc * QC:(sc + 1) * QC],
                        start=(kt == k0), stop=(kt == k1 - 1),
                    )
                if hf == 0:
                    nc.vector.tensor_copy(tgt, ps[:])
                else:
                    nc.vector.tensor_add(tgt, ps[:], tgt)

            chains = [(nm, w_sb, t_sb, g, sc)
                      for nm, w_sb, t_sb in (("k", wk_sb, kT_sb),
                                             ("q", wq_sb, qT_sb))
                      for g in range(2) for sc in range(NQC)]
            for ch in chains:
                emit_proj(*ch, 0)
            for ch in chains:
                if ch[0] == "k" or ch[4] == 0:
                    emit_proj(*ch, 1)

            v_h = v_sb.rearrange("p t (h x) -> p t h x", h=HL)

            def emit_v_tile(st):
                # V natural [s, e_local] + ones column per head
                ps = psmm.tile([128, QC], F32, tag="mm", name=f"pv_v_{st}")
                for kt in range(NDT):
                    nc.tensor.matmul(
                        ps[:, 0:E],
                        xT_sb[:, kt, st * 128:(st + 1) * 128],
                        wv_sb[:, kt, :],
                        start=(kt == 0), stop=(kt == NDT - 1),
                    )
                nc.vector.tensor_copy(
                    v_h[:, st, :, 0:DK],
                    ps[:, 0:E].rearrange("p (h d) -> p h d", h=HL),
                )
                nc.scalar.activation(
                    v_h[:, st, :, DK:DK + 1],
                    zeros[:].rearrange("p (h o) -> p h o", o=1),
                    EXPF, scale=0.0,
                )

            def emit_final(st, ec):
                fp = psmm.tile([128, QC], F32, tag="mm",
                               name=f"f_{st}_{ec}")
                for g in range(2):
                    nc.tensor.matmul(
                        fp[:],
                        oT_sb[:, g, st * 128:(st + 1) * 128],
                        wo_sb[:, g, ec * QC:(ec + 1) * QC],
                        start=(g == 0), stop=(g == 1),
                    )
                fsb = outp.tile([128, QC], F32, tag="fsb")
                nc.vector.tensor_copy(fsb[:], fp[:])
                nc.sync.dma_start(
                    out[st * 128:(st + 1) * 128, ec * QC:(ec + 1) * QC],
                    fsb[:],
                )

            # ---- attention + interleaved final projection -------------
            # Deferred PE filler (remaining Q projections, V projections for
            # the next chunk, finals for the previous chunk) is drained a
            # little at a time inside the attention loop, so the ACT-bound
            # attention stream never stalls behind a large block of
            # projection matmuls.  V tiles get an explicit emission
            # guarantee: all of chunk c+1's V tiles are emitted during
            # chunk c (plus an ensure_v safety net at the use site).
            deferred = [lambda ch=ch: emit_proj(*ch, 1)
                        for ch in chains if ch[0] == "q" and ch[4] > 0]
            for st in range(0, 4):
                emit_v_tile(st)
            v_next = [4]

            def ensure_v(st_needed):
                while v_next[0] <= st_needed:
                    emit_v_tile(v_next[0])
                    v_next[0] += 1

            def drain_one(c):
                if v_next[0] <= 4 * (c + 1) + 3 and c + 1 < NQC:
                    ensure_v(v_next[0])
                elif deferred:
                    deferred.pop(0)()

            for c in range(NQC):
                if c > 0:
                    for st in range(4 * (c - 1), 4 * (c - 1) + 4):
                        for ec in range(2):
                            deferred.append(
                                lambda st=st, ec=ec: emit_final(st, ec))
                for g in range(2):
                    pv_ps = {}
                    n_kt = 4 * c + 4
                    for kt in range(n_kt):
                        diag = kt >= 4 * c
                        j = kt - 4 * c
                        w = min(j, 2) * 128 if diag else 0
                        if diag:
                            ensure_v(kt)
                        st_ps = {}
                        for li in range(2):        # heads 2g, 2g+1 adjacent:
                            r0 = li * 64           # PE row-tiling runs them
                            st_ps[li] = psmm.tile(  # concurrently (K=64 each)
                                [128, QC], F32, tag="mm",
                                name=f"st_{c}_{g}_{kt}_{li}",
                            )
                            nc.tensor.matmul(
                                st_ps[li][:, w:QC],
                                kT_sb[r0:r0 + 64, g, kt * 128:(kt + 1) * 128],
                                qT_sb[r0:r0 + 64, g, c * QC + w:(c + 1) * QC],
                                start=True, stop=True,
                            )
                        for li in range(2):
                            h = 2 * g + li
                            pt = work.tile([128, QC], F32R, tag="pt")
                            nc.scalar.activation(
                                pt[:, w:QC], st_ps[li][:, w:QC], EXPF, scale=SCALE,
                            )
                            if diag:
                                # zero entries with q < k (post-exp)
                                if j < 3:
                                    nc.gpsimd.affine_select(
                                        out=pt[:, j * 128:(j + 1) * 128],
                                        in_=pt[:, j * 128:(j + 1) * 128],
                                        compare_op=mybir.AluOpType.is_ge,
                                        fill=0.0, base=0,
                                        pattern=[[1, 128]], channel_multiplier=-1,
                                    )
                                else:
                                    nc.gpsimd.affine_select(
                                        out=pt[:, 256:512],
                                        in_=pt[:, 256:512],
                                        compare_op=mybir.AluOpType.is_ge,
                                        fill=0.0, base=-128,
                                        pattern=[[1, 256]], channel_multiplier=-1,
                                    )
                            if kt == 0:
                                pv_ps[li] = pspv.tile([128, QC], F32, tag="pv",
                                                      name=f"pv_{c}_{g}_{li}")
                            nc.tensor.matmul(
                                pv_ps[li][0:DK + 1, w:QC],
                                v_sb[:, kt, h * (DK + 1):(h + 1) * (DK + 1)],
                                pt[:, w:QC],
                                start=(kt == 0), stop=(kt == n_kt - 1),
                            )
                        if c == 0 or kt % 2 == 1:
                            drain_one(c)
                    # normalize: oT[head rows, c] = pv[0:64] * (1/pv[64])
                    for li in range(2):
                        r0 = li * 64
                        rc = norm.tile([1, QC], F32, tag="rc")
                        nc.vector.reciprocal(rc[0:1, :], pv_ps[li][DK:DK + 1, :])
                        rbc = norm.tile([64, QC], F32, tag="rbc")
                        nc.gpsimd.partition_broadcast(rbc[:], rc[0:1, :])
                        nc.vector.tensor_mul(
                            oT_sb[r0:r0 + 64, g, c * QC:(c + 1) * QC],
                            pv_ps[li][0:DK, :],
                            rbc[:],
                        )
            while deferred:
                deferred.pop(0)()
            for st in range(12, 16):
                for ec in range(2):
                    emit_final(st, ec)

    nc.compile()
    return nc


_NC = None


def _get_nc():
    global _NC
    if _NC is None:
        _NC = _build_nc()
    return _NC


def _in_maps(x, Wq, Wk, Wv, Wo):
    x, Wq, Wk, Wv, Wo = (np.asarray(a, dtype=np.float32) for a in (x, Wq, Wk, Wv, Wo))
    maps = []
    for c in range(8):
        b, hg = divmod(c, 4)
        sl = slice(hg * E, (hg + 1) * E)
        maps.append({
            "xT": np.ascontiguousarray(x[b].T),
            "wqT": np.ascontiguousarray(Wq[sl].T),
            "wkT": np.ascontiguousarray(Wk[sl].T),
            "wvT": np.ascontiguousarray(Wv[sl].T),
            "woT": np.ascontiguousarray(Wo[:, sl].T),
        })
    return maps


def kernel(x, Wq, Wk, Wv, Wo, _trace=False, _trace_kwargs=None):
    nc = _get_nc()
    maps = _in_maps(x, Wq, Wk, Wv, Wo)
    res = run_bass_kernel_spmd(
        nc, maps, core_ids=list(range(8)),
        trace=_trace, **(_trace_kwargs or {}),
    )
    outs = [res.results[c]["out"] for c in range(8)]
    full = np.stack([
        outs[0] + outs[1] + outs[2] + outs[3],
        outs[4] + outs[5] + outs[6] + outs[7],
    ]).astype(np.float32)
    if _trace:
        return full, res
    return full
